# revision 1
# baseline (speedup 1.0000x reference)
"""Trainium2 Bass kernel for nn_Attention2D (sparse_attention).

Self-contained: takes FULL unsharded inputs, shards data-parallel over the
leading (n_rays) axis across 8 NeuronCores, runs a fused Bass/Tile kernel per
core, gathers the full output.

Math (validated against the jax reference to ~2.6e-6 abs):
  s cancels in kh - qh, so with host-precomputed
    A_k = Wk.T@attn_w1, A_q = Wq.T@attn_w1, P_a = pos_w2@attn_w1,
    c_z = pos_b2@attn_w1 + attn_b1
  the attn-MLP hidden is h1 = relu(k@A_k - q@A_q + hpos@P_a + c_z) with
  hpos = relu(pos@pos_w1 + pos_b1).  The mask is carried through the pipeline
  as an extra matmul row (relu(m)=m for m in {0,1}); masked tokens get their
  h1 clipped to 0 via a +50*(m-1) rank-1 term, and the logits get a +50*m
  rank-1 shift so that after exp(logit-50) masked entries are exp(-50)≈2e-22
  (nonzero => all-masked rays reproduce the reference's uniform softmax).
  u = vh + p (its bias s+pos_b2 is folded into the output bias since softmax
  weights sum to 1), x = (sum_v u*e)/(sum_v e), out = x@out_w + out_b'.

Layout: all on-chip activations are feature-major [channel, token]; the host
pre-transposes k/q/pos/mask into per-core contiguous arrays so every DMA is
dense, and un-transposes the [channel-major] output at the end.
"""

import numpy as np
import ml_dtypes

BF16 = ml_dtypes.bfloat16
DIM, HID, B, N, V = 64, 8, 1024, 64, 8
NCORES = 8
B_C = B // NCORES          # 128 b-rows per core
R_C = B_C * N              # 8192 rays per core
T_C = R_C * V              # 65536 view-tokens per core
TILE_T = 1024              # tokens per tile
HT = TILE_T // 2           # 512  (half-tile tokens; L1 free size)
QT = TILE_T // 4           # 256  (quarter-tile tokens; L2 free size)
RH = TILE_T // 16          # 64   (rays per half-tile)
NT_FULL = T_C // TILE_T    # 64 tiles per core
GRP = 16                   # tiles per pm/out DMA group
M_SHIFT = 50.0             # logit shift for masked-softmax trick
CLIP = 50.0                # relu clipping magnitude for masked tokens

# consts tensor column layout
C_WVT, C_AK, C_AQN, C_WP5, C_WHQ, C_PW2, C_W3, C_OW = 0, 64, 96, 128, 160, 192, 256, 320
C_BHP, C_BH1, C_BOUT, C_BEXP = 384, 385, 386, 387
CW = 388

_PROG_CACHE: dict = {}


# ----------------------------------------------------------------------------
# host-side preparation
# ----------------------------------------------------------------------------

def _f32(x):
    return np.ascontiguousarray(np.asarray(x), dtype=np.float32)


def make_consts(inputs) -> np.ndarray:
    """Build the [128, CW] constants array (shared by all cores)."""
    eid = int(np.asarray(inputs["embed_id1"]))
    Wq = _f32(inputs["q_tbl"])[eid].reshape(DIM, DIM)
    Wk = _f32(inputs["k_tbl"])[eid].reshape(DIM, DIM)
    Wv = _f32(inputs["v_tbl"])[eid].reshape(DIM, DIM)
    pos_w1, pos_b1 = _f32(inputs["pos_w1"]), _f32(inputs["pos_b1"])
    pos_w2, pos_b2 = _f32(inputs["pos_w2"]), _f32(inputs["pos_b2"])
    attn_w1, attn_b1 = _f32(inputs["attn_w1"]), _f32(inputs["attn_b1"])
    attn_w2, attn_b2 = _f32(inputs["attn_w2"]), _f32(inputs["attn_b2"])
    out_w, out_b = _f32(inputs["out_w"]), _f32(inputs["out_b"])
    str_w, str_b = _f32(inputs["str_w"]), _f32(inputs["str_b"])
    strength = _f32(inputs["strength"])

    s = strength @ str_w + str_b                  # [64]
    A_k = Wk.T @ attn_w1                          # [64, 8]
    A_q = Wq.T @ attn_w1                          # [64, 8]
    P_a = pos_w2 @ attn_w1                        # [8, 8]
    c_z = pos_b2 @ attn_w1 + attn_b1              # [8]
    sb2 = s + pos_b2                              # [64]
    out_b_p = sb2 @ out_w + out_b                 # [64]

    C = np.zeros((128, CW), np.float32)
    # Wv.T stacked for both halves (lhsT of u matmul: [in-chan, out-chan])
    C[0:64, C_WVT:C_WVT + 64] = Wv.T
    C[64:128, C_WVT:C_WVT + 64] = Wv.T
    # combined K=128 lhsT for the merged kA-qA matmul:
    # C_AK block pairs with kqA tiles (k-half-A rows 0-63, q-bcast rows 64-127)
    # C_AQN block pairs with kqB tiles (q-bcast rows 0-63, k-half-B rows 64-127)
    C[0:64, C_AK:C_AK + 8] = A_k
    C[64:128, C_AK:C_AK + 8] = -A_q
    C[0:64, C_AQN:C_AQN + 8] = -A_q
    C[64:128, C_AQN:C_AQN + 8] = A_k
    for qq in range(4):
        r = 32 * qq
        # pos-MLP stage1 lhsT [5, 32]: rows 0-3 pos_w1 -> cols 0:8 ; mask row
        # 4 -> col 8 (carries mask into hpos row 8)
        C[r:r + 4, C_WP5:C_WP5 + 8] = pos_w1
        C[r + 4, C_WP5 + 8] = 1.0
        # z_pre stage lhsT [9, 32]: rows 0-7 = P_a -> cols 0:8, mask row 8 ->
        # +CLIP on all 9 outputs
        C[r:r + 8, C_WHQ:C_WHQ + 8] = P_a
        C[r + 8, C_WHQ:C_WHQ + 10] = CLIP
        # pos_w2 [8, 64] for u accumulation
        C[r:r + 8, C_PW2:C_PW2 + 64] = pos_w2
        # W3 [10, 64]: attn_w2 rows + bias row + exact +50 shift row
        C[r:r + 8, C_W3:C_W3 + 64] = attn_w2
        C[r + 8, C_W3:C_W3 + 64] = attn_b2
        C[r + 9, C_W3:C_W3 + 64] = M_SHIFT
        # biases (per-partition vectors)
        C[r:r + 8, C_BHP] = pos_b1
        C[r + 8, C_BHP] = 0.0
        C[r:r + 8, C_BH1] = c_z - CLIP
        C[r + 8, C_BH1] = 1.0 - CLIP
        C[r + 9, C_BH1] = 1.0 - CLIP
    # out_w stacked; out bias per channel stacked
    for h in range(2):
        C[64 * h:64 * h + 64, C_OW:C_OW + 64] = out_w
        C[64 * h:64 * h + 64, C_BOUT] = out_b_p
    C[:, C_BEXP] = -M_SHIFT
    return C


def prep_core(q, k, pos, mask_f, core, nt=NT_FULL):
    """Per-core transposed contiguous arrays. q/k/pos/mask_f are full arrays."""
    ntok = nt * TILE_T
    nray = ntok // V
    b0 = core * B_C
    kc = _f32(k[b0:b0 + B_C]).reshape(T_C, DIM)[:ntok]
    qc = _f32(q[b0:b0 + B_C]).reshape(R_C, DIM)[:nray]
    pc = _f32(pos[b0:b0 + B_C]).reshape(T_C, 4)[:ntok]
    mc = mask_f[b0:b0 + B_C].reshape(T_C)[:ntok]

    # k channel-major halves + per-view-replicated q, combined per half so the
    # merged K=128 kA-qA matmul can stream one tile:
    #   kqA rows 0-63 = k-half-A channels, rows 64-127 = q-half-A repeated x8
    #   kqB rows 0-63 = q-half-B repeated x8, rows 64-127 = k-half-B channels
    kT = kc.reshape(nt, 2, HT, DIM).transpose(1, 3, 0, 2).reshape(128, nt * HT)
    qT = qc.reshape(nt, 2, RH, DIM).transpose(1, 3, 0, 2).reshape(128, nt * RH)
    qrep = np.repeat(qT, V, axis=1)              # [128, nt*HT]
    kqA = np.ascontiguousarray(
        np.concatenate([kT[0:64], qrep[0:64]], axis=0).astype(BF16))
    kqB = np.ascontiguousarray(
        np.concatenate([qrep[64:128], kT[64:128]], axis=0).astype(BF16))
    # posm [20, nt*QT]: row qq*5+e (e<4: pos feat, e=4: mask), col t*QT+j
    pm4 = pc.reshape(nt, 4, QT, 4).transpose(1, 3, 0, 2)        # [4(qq),4(e),nt,QT]
    m4 = mc.reshape(nt, 4, QT).transpose(1, 0, 2)               # [4(qq),nt,QT]
    posm = np.ascontiguousarray(
        np.concatenate([pm4, m4[:, None]], axis=1).reshape(20, nt * QT)
        .astype(BF16))
    return {"kqA": kqA, "kqB": kqB, "posm": posm}


def unprep_out(outT, nt=NT_FULL):
    """outT [128, nt*RH] channel-major -> [nt*2*RH, 64] token-major."""
    v = outT.reshape(2, 64, nt, RH).transpose(2, 0, 3, 1)
    return np.ascontiguousarray(v.reshape(nt * 2 * RH, DIM))


# ----------------------------------------------------------------------------
# device program
# ----------------------------------------------------------------------------

def build_program(nt=NT_FULL, nrep=1, skip=""):
    """Build + compile the per-core Bass program (cached)."""
    if (nt, nrep, skip) in _PROG_CACHE:
        return _PROG_CACHE[(nt, nrep, skip)]

    import concourse.bacc as bacc
    import concourse.tile as tile
    import concourse.mybir as mybir

    f32 = mybir.dt.float32
    bf16 = mybir.dt.bfloat16
    nc = bacc.Bacc("TRN2", target_bir_lowering=False, debug=False,
                   enable_asserts=False, num_devices=NCORES)
    kqA_d = nc.dram_tensor("kqA", [128, nt * HT], bf16, kind="ExternalInput").ap()
    kqB_d = nc.dram_tensor("kqB", [128, nt * HT], bf16, kind="ExternalInput").ap()
    posm_d = nc.dram_tensor("posm", [20, nt * QT], bf16, kind="ExternalInput").ap()
    cons_d = nc.dram_tensor("consts", [128, CW], bf16, kind="ExternalInput").ap()
    bias_d = nc.dram_tensor("biasc", [128, 4], f32, kind="ExternalInput").ap()
    outT_d = nc.dram_tensor("outT", [128, nt * RH], f32, kind="ExternalOutput").ap()

    with tile.TileContext(nc) as tc:
        _emit(tc, nc, mybir, kqA_d, kqB_d, posm_d, cons_d, bias_d, outT_d, nt, nrep, skip)
    nc.compile()
    _PROG_CACHE[(nt, nrep, skip)] = nc
    return nc


def _emit(tc, nc, mybir, kqA_d, kqB_d, posm_d, cons_d, bias_d, outT_d, nt, nrep=1, skip_str=""):
    from contextlib import ExitStack
    skip = set(skip_str.split(","))

    f32 = mybir.dt.float32
    Relu = mybir.ActivationFunctionType.Relu
    Exp = mybir.ActivationFunctionType.Exp
    Ident = mybir.ActivationFunctionType.Identity
    mult = mybir.AluOpType.mult
    AX = mybir.AxisListType.X
    grp = min(GRP, nt)
    bf16 = mybir.dt.bfloat16
    r32 = lambda ap: ap


    with ExitStack() as ctx:
        ep = ctx.enter_context
        cpool = ep(tc.tile_pool(name="consts", bufs=1))
        kpool = ep(tc.tile_pool(name="kt", bufs=2))
        pmpool = ep(tc.tile_pool(name="pm", bufs=2))
        qpool = ep(tc.tile_pool(name="qt", bufs=2))
        hpool = ep(tc.tile_pool(name="hid", bufs=3))
        epool = ep(tc.tile_pool(name="east", bufs=2))
        tpool = ep(tc.tile_pool(name="tprod", bufs=2))
        spool = ep(tc.tile_pool(name="small", bufs=4))
        opool = ep(tc.tile_pool(name="ob", bufs=2))
        pp_h = ep(tc.tile_pool(name="ps_h", bufs=1, space="PSUM"))
        pp_z = ep(tc.tile_pool(name="ps_z", bufs=1 if "bufs2" not in skip else 2,
                               space="PSUM"))
        pp_u = ep(tc.tile_pool(name="ps_u", bufs=1, space="PSUM"))
        pp_l = ep(tc.tile_pool(name="ps_l", bufs=2, space="PSUM"))

        cons = cpool.tile([128, CW], bf16, tag="consts")
        nc.sync.dma_start(cons[:], cons_d[:, :])
        biasc = cpool.tile([128, 4], f32, tag="biasc")
        nc.sync.dma_start(biasc[:], bias_d[:, :])
        b_hp = biasc[:, 0:1]
        b_h1 = biasc[:, 1:2]
        b_out = biasc[:, 2:3]
        b_exp = biasc[:, 3:4]

        for rep in range(nrep):
         for g in range((nt + grp - 1) // grp):
            gt = min(grp, nt - g * grp)
            pm = pmpool.tile([128, grp * QT], bf16, tag="pm")
            for qq in range(4):
                nc.sync.dma_start(
                    pm[32 * qq:32 * qq + 5, 0:gt * QT],
                    posm_d[5 * qq:5 * qq + 5, g * grp * QT:g * grp * QT + gt * QT])
            ob = opool.tile([128, grp * RH], f32, tag="ob")

            for ti in range(gt):
                t = g * grp + ti
                if ti % 8 == 0:
                    kqa_b = kpool.tile([128, 8 * HT], bf16, tag="kqa")
                    kqb_b = kpool.tile([128, 8 * HT], bf16, tag="kqb")
                    nb = min(8, gt - ti)
                    nc.sync.dma_start(kqa_b[:, 0:nb * HT],
                                      kqA_d[:, t * HT:t * HT + nb * HT])
                    nc.sync.dma_start(kqb_b[:, 0:nb * HT],
                                      kqB_d[:, t * HT:t * HT + nb * HT])
                off = (ti % 8) * HT
                kqa = kqa_b[:, off:off + HT]
                kqb = kqb_b[:, off:off + HT]

                pmt = pm[:, ti * QT:(ti + 1) * QT]

                # ---- pos-MLP stage 1 (+ mask carried into hpos row 8) ----
                hpos_ps = pp_h.tile([128, QT], f32, tag="hps")
                for qq in range(4):
                    r = 32 * qq
                    nc.tensor.matmul(
                        hpos_ps[r:r + 32, :], r32(cons[r:r + 5, C_WP5:C_WP5 + 32]),
                        r32(pmt[r:r + 5, :]), start=True, stop=True,
                        tile_position=(r, r), skip_group_check=True)
                hpos = hpool.tile([128, QT], bf16, tag="hpos")
                nc.scalar.activation(hpos[:], hpos_ps[:], Relu, bias=b_hp)

                # ---- z_pre accumulation: (kA - qA) via one K=128 matmul ----
                z_ps = pp_z.tile([128, QT], f32, tag="zps")
                for qq in range(4):
                    r, h, f = 32 * qq, qq // 2, qq % 2
                    kq = kqa if h == 0 else kqb
                    cblk = C_AK if h == 0 else C_AQN
                    nc.tensor.matmul(
                        z_ps[r:r + 32, :], r32(cons[:, cblk:cblk + 32]),
                        r32(kq[:, f * QT:(f + 1) * QT]),
                        start=True, stop=False, tile_position=(0, r),
                        skip_group_check=True)
                for qq in range(4):
                    r = 32 * qq
                    nc.tensor.matmul(
                        z_ps[r:r + 32, :], r32(cons[r:r + 9, C_WHQ:C_WHQ + 32]),
                        r32(hpos[r:r + 9, :]), start=False, stop=True,
                        tile_position=(r, r), skip_group_check=True)
                h1 = hpool.tile([128, QT], bf16, tag="h1")
                nc.scalar.activation(h1[:], z_ps[:], Relu, bias=b_h1)

                # ---- logits = h1m @ [attn_w2; attn_b2; 50] ----
                # two PSUM banks (f=0 at cols 0:256, f=1 at cols 512:768) so
                # concurrent row-group matmuls never drain into the same
                # (partition-range, bank) pair -- that combination hangs HW.
                lg_ps = pp_l.tile([128, 2 * HT], f32, tag="lps")
                for qq in range(4):
                    r, h, f = 32 * qq, qq // 2, qq % 2
                    nc.tensor.matmul(
                        lg_ps[64 * h:64 * h + 64, f * HT:f * HT + QT],
                        r32(cons[r:r + 10, C_W3:C_W3 + 64]), r32(h1[r:r + 10, :]),
                        start=True, stop=True, tile_position=(r, 64 * h),
                        skip_group_check=True)

                # ---- u = k@Wv.T + hpos@pos_w2 ----
                u_ps = pp_u.tile([128, 2 * HT], f32, tag="ups")
                for h in range(2):
                    kq = kqa if h == 0 else kqb
                    for f in range(2):
                        nc.tensor.matmul(
                            u_ps[64 * h:64 * h + 64, f * HT:f * HT + QT],
                            r32(cons[64 * h:64 * h + 64, C_WVT:C_WVT + 64]),
                            r32(kq[64 * h:64 * h + 64, f * QT:(f + 1) * QT]),
                            start=True, stop=False,
                            tile_position=(64 * h, 64 * h),
                            skip_group_check=True)
                for qq in range(4):
                    r, h, f = 32 * qq, qq // 2, qq % 2
                    nc.tensor.matmul(
                        u_ps[64 * h:64 * h + 64, f * HT:f * HT + QT],
                        r32(cons[r:r + 8, C_PW2:C_PW2 + 64]), r32(hpos[r:r + 8, :]),
                        start=False, stop=True, tile_position=(r, 64 * h),
                        skip_group_check=True)

                # ---- softmax over views (groups of 8 along free axis) ----
                east = epool.tile([128, HT], f32, tag="east")
                lg_v = lg_ps[:].rearrange("p (b k) -> p b k", b=2)[:, :, 0:QT]
                if "exp" not in skip:
                    nc.scalar.activation(
                        east[:].rearrange("p (b k) -> p b k", b=2), lg_v, Exp,
                        bias=b_exp)
                if "dve" not in skip:
                    gsum = spool.tile([128, RH], f32, tag="gsum")
                    tp = tpool.tile([128, HT], f32, tag="tp")
                    xr = spool.tile([128, RH], f32, tag="xr")
                    rg = spool.tile([128, RH], f32, tag="rg")
                    xx = spool.tile([128, RH], bf16, tag="xx")
                    nc.vector.reduce_sum(
                        gsum[:], east[:].rearrange("p (r v) -> p r v", v=V), axis=AX)
                    u_v = u_ps[:].rearrange("p (b k) -> p b k", b=2)[:, :, 0:QT]
                    if "tmul" not in skip:
                        nc.vector.tensor_tensor(
                            tp[:].rearrange("p (b k) -> p b k", b=2), u_v,
                            east[:].rearrange("p (b k) -> p b k", b=2), mult)
                    nc.vector.reduce_sum(
                        xr[:], tp[:].rearrange("p (r v) -> p r v", v=V), axis=AX)
                    if "recip" not in skip:
                        nc.vector.reciprocal_approx_fast(rg[:], gsum[:])
                    nc.vector.tensor_tensor(xx[:], xr[:], rg[:], mult)

                # ---- out = x @ out_w + out_b' (channel-major) ----
                if "dve" in skip:
                    xx = spool.tile([128, RH], bf16, tag="xx")
                    src_e = east[:, 0:RH] if "exp" not in skip else h1[:, 0:RH]
                    nc.vector.tensor_copy(xx[:], src_e)
                # out-MM uses lg_ps's unused bank-0 columns: its writers are
                # same-position as the z3 matmuls there (serial, hazard-free),
                # and the freed banks double-buffer lg_ps.
                o_ps = lg_ps[:, QT:QT + RH]
                for h in range(2):
                    nc.tensor.matmul(
                        o_ps[64 * h:64 * h + 64, :],
                        cons[64 * h:64 * h + 64, C_OW:C_OW + 64],
                        xx[64 * h:64 * h + 64, :], start=True, stop=True,
                        tile_position=(64 * h, 64 * h), skip_group_check=True)
                nc.scalar.activation(ob[:, ti * RH:(ti + 1) * RH], o_ps[:],
                                     Ident, bias=b_out)

            nc.sync.dma_start(
                outT_d[:, g * grp * RH:g * grp * RH + gt * RH],
                ob[:, 0:gt * RH])


# ----------------------------------------------------------------------------
# entry point
# ----------------------------------------------------------------------------

def kernel(q, k, pos, strength, q_tbl, k_tbl, v_tbl,
           pos_w1, pos_b1, pos_w2, pos_b2,
           attn_w1, attn_b1, attn_w2, attn_b2,
           out_w, out_b, str_w, str_b, mask, embed_id1) -> np.ndarray:
    from concourse.bass_utils import run_bass_kernel_spmd

    inputs = dict(q=q, k=k, pos=pos, strength=strength, q_tbl=q_tbl,
                  k_tbl=k_tbl, v_tbl=v_tbl, pos_w1=pos_w1, pos_b1=pos_b1,
                  pos_w2=pos_w2, pos_b2=pos_b2, attn_w1=attn_w1,
                  attn_b1=attn_b1, attn_w2=attn_w2, attn_b2=attn_b2,
                  out_w=out_w, out_b=out_b, str_w=str_w, str_b=str_b,
                  mask=mask, embed_id1=embed_id1)
    nc = build_program(NT_FULL)
    consts_f = make_consts(inputs)
    consts = consts_f.astype(BF16)
    biasc = np.ascontiguousarray(
        consts_f[:, [C_BHP, C_BH1, C_BOUT, C_BEXP]], dtype=np.float32)
    mask_f = np.asarray(mask).astype(np.float32)
    in_maps = []
    for c in range(NCORES):
        m = prep_core(inputs["q"], inputs["k"], inputs["pos"], mask_f, c)
        m["consts"] = consts
        m["biasc"] = biasc
        in_maps.append(m)
    res = run_bass_kernel_spmd(nc, in_maps, core_ids=list(range(NCORES)))
    out = np.empty((B * N, DIM), np.float32)
    for c in range(NCORES):
        out[c * R_C:(c + 1) * R_C] = unprep_out(res.results[c]["outT"])
    return out.reshape(B, N, DIM)



# revision 3
# speedup vs baseline: 1.6415x; 1.6415x over previous
"""Trainium2 Bass kernel for nn_Attention2D (sparse_attention), v2.

Data-parallel over rays across 8 cores. Per core, 64 tiles of 1024
view-tokens. All activations feature-major. Within a tile, tokens are
grouped into 8 blocks of 128 tokens (16 rays x 8 views); blocks are paired
(pair i = blocks 2i, 2i+1) so K=64-deep matmuls can stack two blocks into
the 128-partition dim (block-diagonal lhsT), halving the charged output
free size versus a flat layout.

Math (validated in sim_check.py, rel err 6e-7 fp32):
  s cancels in kh - qh; with A_k = Wk.T@attn_w1, A_q = Wq.T@attn_w1,
  P_a = pos_w2@attn_w1, c_z = pos_b2@attn_w1 + attn_b1 the attn-MLP hidden is
  h1 = relu(k@A_k - q@A_q + hpos@P_a + c_z) with hpos = relu(pos@w1+b1).
  Mask rides as hpos channel 8 (ones as channel 9); masked tokens get h1
  clipped to 0 via +CLIP*(m-1), logits get +M_SHIFT*(m-1) so masked
  e = exp(b2-50) ~ 2e-22 (all-masked rays stay uniform). u = k@Wv.T +
  hpos@pos_w2 (its bias s+pos_b2 folds into the output bias).
  x = (sum_v u*e)/(sum_v e); out = x@out_w + b_out'.

Softmax tail: exp on Act; u*e split DVE/GpSimd; the two view-sums run as one
fused pairwise add-tree over the concatenated [e|u*e] fp16 tile (TT ops get
the 2x DVE mode); reciprocal_approx_fast + mult finish.
"""

import numpy as np
import ml_dtypes

BF16 = ml_dtypes.bfloat16
FP16 = np.float16
DIM, HID, B, N, V = 64, 8, 1024, 64, 8
NCORES = 8
B_C = B // NCORES
R_C = B_C * N              # 8192 rays per core
T_C = R_C * V              # 65536 view-tokens per core
TILE = 1024
NT_FULL = T_C // TILE      # 64 tiles
GRP = 16                   # tiles per grouped q/posm/out DMA
KCH = 8                    # tiles per k8 DMA chunk
M_SHIFT = 8.0
CLIP = 8.0
MULT_D = 224               # u*e columns done on DVE (rest on GpSimd)

# consts tensor column layout ([128, CW] bf16)
C_ST1, C_ZP, C_ZK, C_ZQ, C_W3, C_UP, C_WV, C_OW = (
    0, 128, 256, 288, 320, 832, 1344, 1472)
CW = 1600

_PROG_CACHE: dict = {}


def _f32(x):
    return np.ascontiguousarray(np.asarray(x), dtype=np.float32)


# ----------------------------------------------------------------------------
# host-side preparation
# ----------------------------------------------------------------------------

def make_consts(inputs):
    """[128, CW] bf16 consts + [128, 2] f32 biases (b_exp, b_out)."""
    eid = int(np.asarray(inputs["embed_id1"]))
    Wq = _f32(inputs["q_tbl"])[eid].reshape(DIM, DIM)
    Wk = _f32(inputs["k_tbl"])[eid].reshape(DIM, DIM)
    Wv = _f32(inputs["v_tbl"])[eid].reshape(DIM, DIM)
    pos_w1, pos_b1 = _f32(inputs["pos_w1"]), _f32(inputs["pos_b1"])
    pos_w2, pos_b2 = _f32(inputs["pos_w2"]), _f32(inputs["pos_b2"])
    attn_w1, attn_b1 = _f32(inputs["attn_w1"]), _f32(inputs["attn_b1"])
    attn_w2, attn_b2 = _f32(inputs["attn_w2"]), _f32(inputs["attn_b2"])
    out_w, out_b = _f32(inputs["out_w"]), _f32(inputs["out_b"])

    s = _f32(inputs["strength"]) @ _f32(inputs["str_w"]) + _f32(inputs["str_b"])
    A_k = Wk.T @ attn_w1                          # [64, 8]
    A_q = Wq.T @ attn_w1
    P_a = pos_w2 @ attn_w1                        # [8, 8]
    c_z = pos_b2 @ attn_w1 + attn_b1              # [8]
    b_out = (s + pos_b2) @ out_w + out_b          # [64]

    C = np.zeros((128, CW), np.float32)
    # stage-1 lhsT [48, 128]: 8 diag blocks of [6, 16]
    for b in range(8):
        r, c = 6 * b, 16 * b
        C[r:r + 4, C_ST1 + c:C_ST1 + c + 8] = pos_w1
        C[r + 5, C_ST1 + c:C_ST1 + c + 8] = pos_b1
        C[r + 4, C_ST1 + c + 8] = 1.0             # mask carry
        C[r + 5, C_ST1 + c + 9] = 1.0             # ones carry
    # zp lhsT [128, 128]: 8 diag blocks of [16, 16]
    for b in range(8):
        r = 16 * b
        C[r:r + 8, C_ZP + r:C_ZP + r + 8] = P_a
        C[r + 8, C_ZP + r:C_ZP + r + 8] = CLIP
        C[r + 8, C_ZP + r + 8] = 1.0
        C[r + 9, C_ZP + r:C_ZP + r + 8] = c_z - CLIP
        C[r + 9, C_ZP + r + 9] = 1.0
    # z1k / zq lhsT [128, 32]: 2 diag blocks of [64, 16]
    for sub in range(2):
        r, c = 64 * sub, 16 * sub
        C[r:r + 64, C_ZK + c:C_ZK + c + 8] = A_k
        C[r:r + 64, C_ZQ + c:C_ZQ + c + 8] = -A_q
    # W3 / UP lhsT: 4 K-padded [128, 128] blocks (pair i rows at 32i)
    for i in range(4):
        for sub in range(2):
            r, c = 32 * i + 16 * sub, 128 * i + 64 * sub
            C[r:r + 8, C_W3 + c:C_W3 + c + 64] = attn_w2
            C[r + 8, C_W3 + c:C_W3 + c + 64] = M_SHIFT
            C[r + 9, C_W3 + c:C_W3 + c + 64] = attn_b2
            C[r:r + 8, C_UP + c:C_UP + c + 64] = pos_w2
    # Wv.T / out_w [128, 128]: 2 diag blocks of [64, 64]
    for sub in range(2):
        r = 64 * sub
        C[r:r + 64, C_WV + r:C_WV + r + 64] = Wv.T
        C[r:r + 64, C_OW + r:C_OW + r + 64] = out_w

    biasf = np.zeros((128, 2), np.float32)
    biasf[:, 0] = -M_SHIFT
    for sub in range(2):
        biasf[64 * sub:64 * sub + 64, 1] = b_out
    return C.astype(BF16), biasf


def prep_core(q, k, pos, mask_f, core):
    """Per-core transposed contiguous arrays (bf16)."""
    b0 = core * B_C
    kc = _f32(k[b0:b0 + B_C]).reshape(T_C, DIM).astype(BF16)
    qc = _f32(q[b0:b0 + B_C]).reshape(R_C, DIM).astype(BF16)
    pc = _f32(pos[b0:b0 + B_C]).reshape(T_C, 4)
    mc = mask_f[b0:b0 + B_C].reshape(T_C)

    # k8[sub*64+ch, t*512 + i*128 + j] = k[t*1024 + (2i+sub)*128 + j, ch]
    k8 = np.ascontiguousarray(
        kc.reshape(NT_FULL, 4, 2, 128, DIM).transpose(2, 4, 0, 1, 3)
        .reshape(128, NT_FULL * 512))
    # q8r[sub*64+ch, t*512 + i*128 + jr*8 + v] = q[t*128 + (2i+sub)*16 + jr, ch]
    q8 = qc.reshape(NT_FULL, 4, 2, 16, DIM).transpose(2, 4, 0, 1, 3)
    q8 = np.ascontiguousarray(np.broadcast_to(
        q8.reshape(2, DIM, NT_FULL, 4, 16, 1),
        (2, DIM, NT_FULL, 4, 16, V)).reshape(128, NT_FULL * 512))
    # posm[b*6+e, t*128 + j]: e 0-3 pos, 4 mask, 5 ones
    pm = np.empty((8, 6, NT_FULL, 128), np.float32)
    pm[:, 0:4] = pc.reshape(NT_FULL, 8, 128, 4).transpose(1, 3, 0, 2)
    pm[:, 4] = mc.reshape(NT_FULL, 8, 128).transpose(1, 0, 2)
    pm[:, 5] = 1.0
    posm = np.ascontiguousarray(pm.reshape(48, NT_FULL * 128).astype(BF16))
    return {"k8": k8, "q8": q8, "posm": posm}


def unprep_out(outT):
    """outT [128, NT*64] -> [R_C, 64] ray-major."""
    v = outT.reshape(2, DIM, NT_FULL, 4, 16).transpose(2, 3, 0, 4, 1)
    return np.ascontiguousarray(v.reshape(R_C, DIM))


# ----------------------------------------------------------------------------
# device program
# ----------------------------------------------------------------------------

def build_program(nt=NT_FULL, nrep=1):
    if (nt, nrep) in _PROG_CACHE:
        return _PROG_CACHE[(nt, nrep)]

    import concourse.bacc as bacc
    import concourse.tile as tile
    import concourse.mybir as mybir

    f32 = mybir.dt.float32
    bf16 = mybir.dt.bfloat16
    nc = bacc.Bacc("TRN2", target_bir_lowering=False, debug=False,
                   enable_asserts=False, num_devices=NCORES)
    k8_d = nc.dram_tensor("k8", [128, nt * 512], bf16, kind="ExternalInput").ap()
    q8_d = nc.dram_tensor("q8", [128, nt * 512], bf16, kind="ExternalInput").ap()
    posm_d = nc.dram_tensor("posm", [48, nt * 128], bf16, kind="ExternalInput").ap()
    cons_d = nc.dram_tensor("consts", [128, CW], bf16, kind="ExternalInput").ap()
    bias_d = nc.dram_tensor("biasf", [128, 2], f32, kind="ExternalInput").ap()
    outT_d = nc.dram_tensor("outT", [128, nt * 64], f32, kind="ExternalOutput").ap()

    with tile.TileContext(nc) as tc:
        _emit(tc, nc, mybir, k8_d, q8_d, posm_d, cons_d, bias_d, outT_d, nt, nrep)
    nc.compile()
    _PROG_CACHE[(nt, nrep)] = nc
    return nc


def _emit(tc, nc, mybir, k8_d, q8_d, posm_d, cons_d, bias_d, outT_d, nt, nrep=1):
    from contextlib import ExitStack

    f32 = mybir.dt.float32
    bf16 = mybir.dt.bfloat16
    fp16 = mybir.dt.float16
    Exp = mybir.ActivationFunctionType.Exp
    Copy = mybir.ActivationFunctionType.Copy
    Relu = mybir.ActivationFunctionType.Relu
    Ident = mybir.ActivationFunctionType.Identity
    add = mybir.AluOpType.add
    mult = mybir.AluOpType.mult
    grp = min(GRP, nt)
    kch = min(KCH, nt)

    with ExitStack() as ctx:
        ep = ctx.enter_context
        cpool = ep(tc.tile_pool(name="consts", bufs=1))
        kpool = ep(tc.tile_pool(name="kt", bufs=2))
        qpool = ep(tc.tile_pool(name="qt", bufs=2))
        pmpool = ep(tc.tile_pool(name="pm", bufs=2))
        spool = ep(tc.tile_pool(name="sb", bufs=4))
        tpool = ep(tc.tile_pool(name="tree", bufs=4))
        xpool = ep(tc.tile_pool(name="xs", bufs=4))
        opool = ep(tc.tile_pool(name="ob", bufs=2))
        pp_hp = ep(tc.tile_pool(name="ps_hp", bufs=1, space="PSUM"))
        pp_z = ep(tc.tile_pool(name="ps_z", bufs=2, space="PSUM"))
        pp_lg = ep(tc.tile_pool(name="ps_lg", bufs=2, space="PSUM"))
        pp_o = ep(tc.tile_pool(name="ps_o", bufs=1, space="PSUM"))
        pp_u = ep(tc.tile_pool(name="ps_u", bufs=2, space="PSUM"))

        cons = cpool.tile([128, CW], bf16, tag="consts")
        nc.sync.dma_start(cons[:], cons_d[:, :])
        biasf = cpool.tile([128, 2], f32, tag="biasf")
        nc.sync.dma_start(biasf[:], bias_d[:, :])
        b_exp = biasf[:, 0:1]
        b_out = biasf[:, 1:2]

        lT1 = cons[0:48, C_ST1:C_ST1 + 128]
        lZP = cons[:, C_ZP:C_ZP + 128]
        lZK = cons[:, C_ZK:C_ZK + 32]
        lZQ = cons[:, C_ZQ:C_ZQ + 32]
        lWV = cons[:, C_WV:C_WV + 128]
        lOW = cons[:, C_OW:C_OW + 128]

        for rep in range(nrep):
         def front(t, kt, qt, hpos):
            """Stages up to exp: returns (lg, u, et) tiles for the tail."""
            z8_t = pp_z.tile([128, 128], f32, tag="z8")
            z8 = z8_t[:]
            for i in range(4):
                nc.tensor.matmul(
                    z8[32 * i:32 * i + 32, :], lZK,
                    kt[:, 128 * i:128 * i + 128], start=True, stop=False,
                    tile_position=(0, 32 * i), skip_group_check=True)
            for i in range(4):
                nc.tensor.matmul(
                    z8[32 * i:32 * i + 32, :], lZQ,
                    qt[:, 128 * i:128 * i + 128], start=False, stop=False,
                    tile_position=(0, 32 * i), skip_group_check=True)
            nc.tensor.matmul(z8, lZP, hpos, start=False, stop=True,
                             tile_position=(0, 0), skip_group_check=True)
            h1 = spool.tile([128, 128], bf16, tag="h1")
            nc.vector.tensor_scalar_max(h1[:], z8, 0.0)

            lg = pp_lg.tile([128, 512], f32, tag="lg")
            for i in range(4):
                nc.tensor.matmul(
                    lg[:, 128 * i:128 * i + 128],
                    cons[:, C_W3 + 128 * i:C_W3 + 128 * i + 128],
                    h1[:], start=True, stop=True,
                    tile_position=(0, 0), skip_group_check=True)

            u = pp_u.tile([128, 512], f32, tag="u")
            nc.tensor.matmul(u[:], lWV, kt, start=True, stop=False,
                             tile_position=(0, 0), skip_group_check=True)
            for i in range(4):
                nc.tensor.matmul(
                    u[:, 128 * i:128 * i + 128],
                    cons[:, C_UP + 128 * i:C_UP + 128 * i + 128],
                    hpos, start=False, stop=True,
                    tile_position=(0, 0), skip_group_check=True)

            et = spool.tile([128, 1024], fp16, tag="et")
            nc.scalar.activation(et[:, 0:512], lg[:], Exp, bias=b_exp)
            return lg, u, et

         def tail(ti_ob, ob, lg, u, et):
            with nc.allow_low_precision(reason="fp16 softmax tail"):
                nc.vector.tensor_tensor(
                    et[:, 512:1024], u[:, 0:512], et[:, 0:512], mult)
                etv = et[:].rearrange("p (gp v) -> p gp v", v=8)
                t1 = tpool.tile([128, 512], fp16, tag="t1")
                t1v = t1[:].rearrange("p (gp v) -> p gp v", v=4)
                nc.vector.tensor_tensor(t1v, etv[:, :, 0:4], etv[:, :, 4:8], add)
                t2 = tpool.tile([128, 256], fp16, tag="t2")
                t2v = t2[:].rearrange("p (gp v) -> p gp v", v=2)
                nc.gpsimd.tensor_tensor(t2v, t1v[:, :, 0:2], t1v[:, :, 2:4], add)
                dn = tpool.tile([128, 128], f32, tag="dn")
                dnv = dn[:].rearrange("p (gp v) -> p gp v", v=1)
                nc.gpsimd.tensor_tensor(dnv, t2v[:, :, 0:1], t2v[:, :, 1:2], add)
            rden = xpool.tile([128, 64], f32, tag="rden")
            nc.vector.reciprocal_approx_fast(rden[:], dn[:, 0:64])
            x = xpool.tile([128, 64], bf16, tag="x")
            nc.gpsimd.tensor_tensor(x[:], dn[:, 64:128], rden[:], mult)

            o_t = pp_o.tile([128, 64], f32, tag="o")
            o_ps = o_t[:]
            nc.tensor.matmul(o_ps, lOW, x[:], start=True, stop=True,
                             tile_position=(0, 0), skip_group_check=True)
            nc.scalar.activation(ob[:, ti_ob * 64:ti_ob * 64 + 64], o_ps,
                                 Ident, bias=b_out)

         # software-pipelined main loop: front(t+1) is emitted before tail(t)
         pending = None          # (ti_ob, ob, lg, u, et)
         kb = None
         obs = {}
         for t in range(nt):
            g, ti = t // grp, t % grp
            if ti == 0:
                gt = min(grp, nt - g * grp)
                t0 = g * grp
                pass
                pb = pmpool.tile([48, grp * 128], bf16, tag="pb")
                nc.sync.dma_start(pb[:, 0:gt * 128],
                                  posm_d[:, t0 * 128:(t0 + gt) * 128])
                obs[g] = opool.tile([128, grp * 64], f32, tag="ob", name=f"ob{g}")
            if t == 0:
                kb = kpool.tile([128, kch * 512], bf16, tag="kb")
                nc.sync.dma_start(kb[:, 0:min(kch, nt) * 512],
                                  k8_d[:, 0:min(kch, nt) * 512])
                qrb = qpool.tile([128, kch * 512], bf16, tag="qrb")
                nc.sync.dma_start(qrb[:, 0:min(kch, nt) * 512],
                                  q8_d[:, 0:min(kch, nt) * 512])
                kb_next = None
                qrb_next = None
            if t % kch == 1 and t + kch - 1 < nt:
                kb_next = kpool.tile([128, kch * 512], bf16, tag="kb")
                qrb_next = qpool.tile([128, kch * 512], bf16, tag="qrb")
                c0 = (t // kch + 1) * kch
                nb = min(kch, nt - c0)
                nc.sync.dma_start(kb_next[:, 0:nb * 512],
                                  k8_d[:, c0 * 512:(c0 + nb) * 512])
                nc.sync.dma_start(qrb_next[:, 0:nb * 512],
                                  q8_d[:, c0 * 512:(c0 + nb) * 512])
            if t % kch == 0 and t > 0:
                kb = kb_next
                qrb = qrb_next
            kt = kb[:, (t % kch) * 512:(t % kch) * 512 + 512]
            qt = qrb[:, (t % kch) * 512:(t % kch) * 512 + 512]

            if ti % 4 == 0:
                n4 = min(4, gt - ti)
                hp_t = pp_hp.tile([128, 512], f32, tag="hp")
                nc.tensor.matmul(hp_t[:, 0:n4 * 128], lT1,
                                 pb[:, ti * 128:(ti + n4) * 128],
                                 start=True, stop=True,
                                 tile_position=(0, 0), skip_group_check=True)
                hpos4 = spool.tile([128, 512], bf16, tag="hpos4")
                nc.scalar.activation(hpos4[:, 0:n4 * 128], hp_t[:, 0:n4 * 128],
                                     Relu)
            hpos = hpos4[:, (ti % 4) * 128:(ti % 4) * 128 + 128]

            def flush(tdone):
                pg = tdone // grp
                if tdone % grp == grp - 1 or tdone == nt - 1:
                    pgt = min(grp, nt - pg * grp)
                    p0 = pg * grp
                    nc.sync.dma_start(
                        outT_d[:, p0 * 64:(p0 + pgt) * 64],
                        obs.pop(pg)[:, 0:pgt * 64])

            cur = (ti, obs[g], *front(t, kt, qt, hpos))
            if pending is not None:
                tail(*pending)
                flush(t - 1)
            pending = cur
         tail(*pending)
         flush(nt - 1)


# ----------------------------------------------------------------------------
# entry point
# ----------------------------------------------------------------------------

def kernel(q, k, pos, strength, q_tbl, k_tbl, v_tbl,
           pos_w1, pos_b1, pos_w2, pos_b2,
           attn_w1, attn_b1, attn_w2, attn_b2,
           out_w, out_b, str_w, str_b, mask, embed_id1) -> np.ndarray:
    from concourse.bass_utils import run_bass_kernel_spmd

    inputs = dict(q=q, k=k, pos=pos, strength=strength, q_tbl=q_tbl,
                  k_tbl=k_tbl, v_tbl=v_tbl, pos_w1=pos_w1, pos_b1=pos_b1,
                  pos_w2=pos_w2, pos_b2=pos_b2, attn_w1=attn_w1,
                  attn_b1=attn_b1, attn_w2=attn_w2, attn_b2=attn_b2,
                  out_w=out_w, out_b=out_b, str_w=str_w, str_b=str_b,
                  mask=mask, embed_id1=embed_id1)
    nc = build_program(NT_FULL)
    consts, biasf = make_consts(inputs)
    mask_f = np.asarray(mask).astype(np.float32)
    in_maps = []
    for c in range(NCORES):
        m = prep_core(inputs["q"], inputs["k"], inputs["pos"], mask_f, c)
        m["consts"] = consts
        m["biasf"] = biasf
        in_maps.append(m)
    res = run_bass_kernel_spmd(nc, in_maps, core_ids=list(range(NCORES)))
    out = np.empty((B * N, DIM), np.float32)
    for c in range(NCORES):
        out[c * R_C:(c + 1) * R_C] = unprep_out(res.results[c]["outT"])
    return out.reshape(B, N, DIM)


# revision 4
# speedup vs baseline: 1.7720x; 1.0795x over previous
"""Trainium2 Bass kernel for nn_Attention2D (sparse_attention), v2.

Data-parallel over rays across 8 cores. Per core, 64 tiles of 1024
view-tokens. All activations feature-major. Within a tile, tokens are
grouped into 8 blocks of 128 tokens (16 rays x 8 views); blocks are paired
(pair i = blocks 2i, 2i+1) so K=64-deep matmuls can stack two blocks into
the 128-partition dim (block-diagonal lhsT), halving the charged output
free size versus a flat layout.

Math (validated in sim_check.py, rel err 6e-7 fp32):
  s cancels in kh - qh; with A_k = Wk.T@attn_w1, A_q = Wq.T@attn_w1,
  P_a = pos_w2@attn_w1, c_z = pos_b2@attn_w1 + attn_b1 the attn-MLP hidden is
  h1 = relu(k@A_k - q@A_q + hpos@P_a + c_z) with hpos = relu(pos@w1+b1).
  Mask rides as hpos channel 8 (ones as channel 9); masked tokens get h1
  clipped to 0 via +CLIP*(m-1), logits get +M_SHIFT*(m-1) so masked
  e = exp(b2-50) ~ 2e-22 (all-masked rays stay uniform). u = k@Wv.T +
  hpos@pos_w2 (its bias s+pos_b2 folds into the output bias).
  x = (sum_v u*e)/(sum_v e); out = x@out_w + b_out'.

Softmax tail: exp on Act; u*e split DVE/GpSimd; the two view-sums run as one
fused pairwise add-tree over the concatenated [e|u*e] fp16 tile (TT ops get
the 2x DVE mode); reciprocal_approx_fast + mult finish.
"""

import numpy as np
import ml_dtypes

BF16 = ml_dtypes.bfloat16
FP16 = np.float16
DIM, HID, B, N, V = 64, 8, 1024, 64, 8
NCORES = 8
B_C = B // NCORES
R_C = B_C * N              # 8192 rays per core
T_C = R_C * V              # 65536 view-tokens per core
TILE = 1024
NT_FULL = T_C // TILE      # 64 tiles
GRP = 16                   # tiles per grouped q/posm/out DMA
KCH = 8                    # tiles per k8 DMA chunk
M_SHIFT = 8.0
CLIP = 8.0
MULT_D = 224               # u*e columns done on DVE (rest on GpSimd)

# consts tensor column layout ([128, CW] bf16)
C_ST1, C_ZP, C_ZK, C_ZQ, C_W3, C_UP, C_WV, C_OW = (
    0, 128, 256, 288, 320, 832, 1344, 1472)
CW = 1600

_PROG_CACHE: dict = {}


def _f32(x):
    return np.ascontiguousarray(np.asarray(x), dtype=np.float32)


# ----------------------------------------------------------------------------
# host-side preparation
# ----------------------------------------------------------------------------

def make_consts(inputs):
    """[128, CW] bf16 consts + [128, 2] f32 biases (b_exp, b_out)."""
    eid = int(np.asarray(inputs["embed_id1"]))
    Wq = _f32(inputs["q_tbl"])[eid].reshape(DIM, DIM)
    Wk = _f32(inputs["k_tbl"])[eid].reshape(DIM, DIM)
    Wv = _f32(inputs["v_tbl"])[eid].reshape(DIM, DIM)
    pos_w1, pos_b1 = _f32(inputs["pos_w1"]), _f32(inputs["pos_b1"])
    pos_w2, pos_b2 = _f32(inputs["pos_w2"]), _f32(inputs["pos_b2"])
    attn_w1, attn_b1 = _f32(inputs["attn_w1"]), _f32(inputs["attn_b1"])
    attn_w2, attn_b2 = _f32(inputs["attn_w2"]), _f32(inputs["attn_b2"])
    out_w, out_b = _f32(inputs["out_w"]), _f32(inputs["out_b"])

    s = _f32(inputs["strength"]) @ _f32(inputs["str_w"]) + _f32(inputs["str_b"])
    A_k = Wk.T @ attn_w1                          # [64, 8]
    A_q = Wq.T @ attn_w1
    P_a = pos_w2 @ attn_w1                        # [8, 8]
    c_z = pos_b2 @ attn_w1 + attn_b1              # [8]
    b_out = (s + pos_b2) @ out_w + out_b          # [64]

    C = np.zeros((128, CW), np.float32)
    # stage-1 lhsT [48, 128]: 8 diag blocks of [6, 16]
    for b in range(8):
        r, c = 6 * b, 16 * b
        C[r:r + 4, C_ST1 + c:C_ST1 + c + 8] = pos_w1
        C[r + 5, C_ST1 + c:C_ST1 + c + 8] = pos_b1
        C[r + 4, C_ST1 + c + 8] = 1.0             # mask carry
        C[r + 5, C_ST1 + c + 9] = 1.0             # ones carry
    # zp lhsT [128, 128]: 8 diag blocks of [16, 16]
    for b in range(8):
        r = 16 * b
        C[r:r + 8, C_ZP + r:C_ZP + r + 8] = P_a
        C[r + 8, C_ZP + r:C_ZP + r + 8] = CLIP
        C[r + 8, C_ZP + r + 8] = 1.0
        C[r + 9, C_ZP + r:C_ZP + r + 8] = c_z - CLIP
        C[r + 9, C_ZP + r + 9] = 1.0
    # z1k / zq lhsT [128, 32]: 2 diag blocks of [64, 16]
    for sub in range(2):
        r, c = 64 * sub, 16 * sub
        C[r:r + 64, C_ZK + c:C_ZK + c + 8] = A_k
        C[r:r + 64, C_ZQ + c:C_ZQ + c + 8] = -A_q
    # W3 / UP lhsT: 4 K-padded [128, 128] blocks (pair i rows at 32i)
    for i in range(4):
        for sub in range(2):
            r, c = 32 * i + 16 * sub, 128 * i + 64 * sub
            C[r:r + 8, C_W3 + c:C_W3 + c + 64] = attn_w2
            C[r + 8, C_W3 + c:C_W3 + c + 64] = M_SHIFT
            C[r + 9, C_W3 + c:C_W3 + c + 64] = attn_b2
            C[r:r + 8, C_UP + c:C_UP + c + 64] = pos_w2
    # Wv.T / out_w [128, 128]: 2 diag blocks of [64, 64]
    for sub in range(2):
        r = 64 * sub
        C[r:r + 64, C_WV + r:C_WV + r + 64] = Wv.T
        C[r:r + 64, C_OW + r:C_OW + r + 64] = out_w

    biasf = np.zeros((128, 2), np.float32)
    biasf[:, 0] = -M_SHIFT
    for sub in range(2):
        biasf[64 * sub:64 * sub + 64, 1] = b_out
    return C.astype(BF16), biasf


def prep_core(q, k, pos, mask_f, core):
    """Per-core transposed contiguous arrays (bf16)."""
    b0 = core * B_C
    kc = _f32(k[b0:b0 + B_C]).reshape(T_C, DIM).astype(BF16)
    qc = _f32(q[b0:b0 + B_C]).reshape(R_C, DIM).astype(BF16)
    pc = _f32(pos[b0:b0 + B_C]).reshape(T_C, 4)
    mc = mask_f[b0:b0 + B_C].reshape(T_C)

    # k8[sub*64+ch, t*512 + i*128 + j] = k[t*1024 + (2i+sub)*128 + j, ch]
    k8 = np.ascontiguousarray(
        kc.reshape(NT_FULL, 4, 2, 128, DIM).transpose(2, 4, 0, 1, 3)
        .reshape(128, NT_FULL * 512))
    # q8r[sub*64+ch, t*512 + i*128 + jr*8 + v] = q[t*128 + (2i+sub)*16 + jr, ch]
    q8 = qc.reshape(NT_FULL, 4, 2, 16, DIM).transpose(2, 4, 0, 1, 3)
    q8 = np.ascontiguousarray(np.broadcast_to(
        q8.reshape(2, DIM, NT_FULL, 4, 16, 1),
        (2, DIM, NT_FULL, 4, 16, V)).reshape(128, NT_FULL * 512))
    # posm[b*6+e, t*128 + j]: e 0-3 pos, 4 mask, 5 ones
    pm = np.empty((8, 6, NT_FULL, 128), np.float32)
    pm[:, 0:4] = pc.reshape(NT_FULL, 8, 128, 4).transpose(1, 3, 0, 2)
    pm[:, 4] = mc.reshape(NT_FULL, 8, 128).transpose(1, 0, 2)
    pm[:, 5] = 1.0
    posm = np.ascontiguousarray(pm.reshape(48, NT_FULL * 128).astype(BF16))
    return {"k8": k8, "q8": q8, "posm": posm}


def unprep_out(outT):
    """outT [128, NT*64] -> [R_C, 64] ray-major."""
    v = outT.reshape(2, DIM, NT_FULL, 4, 16).transpose(2, 3, 0, 4, 1)
    return np.ascontiguousarray(v.reshape(R_C, DIM))


# ----------------------------------------------------------------------------
# device program
# ----------------------------------------------------------------------------

def build_program(nt=NT_FULL, nrep=1):
    if (nt, nrep) in _PROG_CACHE:
        return _PROG_CACHE[(nt, nrep)]

    import concourse.bacc as bacc
    import concourse.tile as tile
    import concourse.mybir as mybir

    f32 = mybir.dt.float32
    bf16 = mybir.dt.bfloat16
    nc = bacc.Bacc("TRN2", target_bir_lowering=False, debug=False,
                   enable_asserts=False, num_devices=NCORES)
    k8_d = nc.dram_tensor("k8", [128, nt * 512], bf16, kind="ExternalInput").ap()
    q8_d = nc.dram_tensor("q8", [128, nt * 512], bf16, kind="ExternalInput").ap()
    posm_d = nc.dram_tensor("posm", [48, nt * 128], bf16, kind="ExternalInput").ap()
    cons_d = nc.dram_tensor("consts", [128, CW], bf16, kind="ExternalInput").ap()
    bias_d = nc.dram_tensor("biasf", [128, 2], f32, kind="ExternalInput").ap()
    outT_d = nc.dram_tensor("outT", [128, nt * 64], f32, kind="ExternalOutput").ap()

    with tile.TileContext(nc) as tc:
        _emit(tc, nc, mybir, k8_d, q8_d, posm_d, cons_d, bias_d, outT_d, nt, nrep)
    nc.compile()
    _PROG_CACHE[(nt, nrep)] = nc
    return nc


def _emit(tc, nc, mybir, k8_d, q8_d, posm_d, cons_d, bias_d, outT_d, nt, nrep=1):
    from contextlib import ExitStack

    f32 = mybir.dt.float32
    bf16 = mybir.dt.bfloat16
    fp16 = mybir.dt.float16
    Exp = mybir.ActivationFunctionType.Exp
    Copy = mybir.ActivationFunctionType.Copy
    Relu = mybir.ActivationFunctionType.Relu
    Ident = mybir.ActivationFunctionType.Identity
    add = mybir.AluOpType.add
    mult = mybir.AluOpType.mult
    grp = min(GRP, nt)
    kch = min(KCH, nt)

    with ExitStack() as ctx:
        ep = ctx.enter_context
        cpool = ep(tc.tile_pool(name="consts", bufs=1))
        kpool = ep(tc.tile_pool(name="kt", bufs=2))
        qpool = ep(tc.tile_pool(name="qt", bufs=2))
        pmpool = ep(tc.tile_pool(name="pm", bufs=2))
        spool = ep(tc.tile_pool(name="sb", bufs=6))
        tpool = ep(tc.tile_pool(name="tree", bufs=6))
        xpool = ep(tc.tile_pool(name="xs", bufs=6))
        opool = ep(tc.tile_pool(name="ob", bufs=2))
        pp_hp = ep(tc.tile_pool(name="ps_hp", bufs=1, space="PSUM"))
        pp_z = ep(tc.tile_pool(name="ps_z", bufs=1, space="PSUM"))
        pp_lg = ep(tc.tile_pool(name="ps_lg", bufs=1, space="PSUM"))
        pp_o = ep(tc.tile_pool(name="ps_o", bufs=1, space="PSUM"))
        pp_u = ep(tc.tile_pool(name="ps_u", bufs=4, space="PSUM"))

        cons = cpool.tile([128, CW], bf16, tag="consts")
        nc.sync.dma_start(cons[:], cons_d[:, :])
        biasf = cpool.tile([128, 2], f32, tag="biasf")
        nc.sync.dma_start(biasf[:], bias_d[:, :])
        b_exp = biasf[:, 0:1]
        b_out = biasf[:, 1:2]

        lT1 = cons[0:48, C_ST1:C_ST1 + 128]
        lZP = cons[:, C_ZP:C_ZP + 128]
        lZK = cons[:, C_ZK:C_ZK + 32]
        lZQ = cons[:, C_ZQ:C_ZQ + 32]
        lWV = cons[:, C_WV:C_WV + 128]
        lOW = cons[:, C_OW:C_OW + 128]

        for rep in range(nrep):
         def front(t, kt, qt, hpos):
            """Stages up to exp: returns (lg, u, et) tiles for the tail."""
            z8_t = pp_z.tile([128, 128], f32, tag="z8")
            z8 = z8_t[:]
            for i in range(4):
                nc.tensor.matmul(
                    z8[32 * i:32 * i + 32, :], lZK,
                    kt[:, 128 * i:128 * i + 128], start=True, stop=False,
                    tile_position=(0, 32 * i), skip_group_check=True)
            for i in range(4):
                nc.tensor.matmul(
                    z8[32 * i:32 * i + 32, :], lZQ,
                    qt[:, 128 * i:128 * i + 128], start=False, stop=False,
                    tile_position=(0, 32 * i), skip_group_check=True)
            nc.tensor.matmul(z8, lZP, hpos, start=False, stop=True,
                             tile_position=(0, 0), skip_group_check=True)
            h1 = spool.tile([128, 128], bf16, tag="h1")
            nc.vector.tensor_scalar_max(h1[:], z8, 0.0)

            lg = pp_lg.tile([128, 512], f32, tag="lg")
            for i in range(4):
                nc.tensor.matmul(
                    lg[:, 128 * i:128 * i + 128],
                    cons[:, C_W3 + 128 * i:C_W3 + 128 * i + 128],
                    h1[:], start=True, stop=True,
                    tile_position=(0, 0), skip_group_check=True)

            u = pp_u.tile([128, 512], f32, tag="u")
            nc.tensor.matmul(u[:], lWV, kt, start=True, stop=False,
                             tile_position=(0, 0), skip_group_check=True)
            for i in range(4):
                nc.tensor.matmul(
                    u[:, 128 * i:128 * i + 128],
                    cons[:, C_UP + 128 * i:C_UP + 128 * i + 128],
                    hpos, start=False, stop=True,
                    tile_position=(0, 0), skip_group_check=True)

            et = spool.tile([128, 1024], fp16, tag="et")
            nc.scalar.activation(et[:, 0:512], lg[:], Exp, bias=b_exp)
            return lg, u, et

         def tail1(ti_ob, ob, lg, u, et):
            with nc.allow_low_precision(reason="fp16 softmax tail"):
                nc.vector.tensor_tensor(
                    et[:, 512:1024], u[:, 0:512], et[:, 0:512], mult)
                etv = et[:].rearrange("p (gp v) -> p gp v", v=8)
                t1 = tpool.tile([128, 512], fp16, tag="t1")
                t1v = t1[:].rearrange("p (gp v) -> p gp v", v=4)
                nc.vector.tensor_tensor(t1v, etv[:, :, 0:4], etv[:, :, 4:8], add)
                t2 = tpool.tile([128, 256], fp16, tag="t2")
                t2v = t2[:].rearrange("p (gp v) -> p gp v", v=2)
                nc.gpsimd.tensor_tensor(t2v, t1v[:, :, 0:2], t1v[:, :, 2:4], add)
                dn = tpool.tile([128, 128], f32, tag="dn")
                dnv = dn[:].rearrange("p (gp v) -> p gp v", v=1)
                nc.gpsimd.tensor_tensor(dnv, t2v[:, :, 0:1], t2v[:, :, 1:2], add)
            return (ti_ob, ob, dn)

         def tail2(ti_ob, ob, dn):
            rden = xpool.tile([128, 64], f32, tag="rden")
            nc.vector.reciprocal_approx_fast(rden[:], dn[:, 0:64])
            x = xpool.tile([128, 64], bf16, tag="x")
            nc.gpsimd.tensor_tensor(x[:], dn[:, 64:128], rden[:], mult)

            o_t = pp_o.tile([128, 64], f32, tag="o")
            o_ps = o_t[:]
            nc.tensor.matmul(o_ps, lOW, x[:], start=True, stop=True,
                             tile_position=(0, 0), skip_group_check=True)
            nc.scalar.activation(ob[:, ti_ob * 64:ti_ob * 64 + 64], o_ps,
                                 Ident, bias=b_out)

         # software-pipelined main loop: front(t+1) is emitted before tail(t)
         pq = []                 # [(t, front-result or tail1-result)]
         kb = None
         obs = {}
         for t in range(nt):
            g, ti = t // grp, t % grp
            if ti == 0:
                gt = min(grp, nt - g * grp)
                t0 = g * grp
                pass
                pb = pmpool.tile([48, grp * 128], bf16, tag="pb")
                nc.sync.dma_start(pb[:, 0:gt * 128],
                                  posm_d[:, t0 * 128:(t0 + gt) * 128])
                obs[g] = opool.tile([128, grp * 64], f32, tag="ob", name=f"ob{g}")
            if t == 0:
                kb = kpool.tile([128, kch * 512], bf16, tag="kb")
                nc.sync.dma_start(kb[:, 0:min(kch, nt) * 512],
                                  k8_d[:, 0:min(kch, nt) * 512])
                qrb = qpool.tile([128, kch * 512], bf16, tag="qrb")
                nc.sync.dma_start(qrb[:, 0:min(kch, nt) * 512],
                                  q8_d[:, 0:min(kch, nt) * 512])
                kb_next = None
                qrb_next = None
            if t % kch == 1 and t + kch - 1 < nt:
                kb_next = kpool.tile([128, kch * 512], bf16, tag="kb")
                qrb_next = qpool.tile([128, kch * 512], bf16, tag="qrb")
                c0 = (t // kch + 1) * kch
                nb = min(kch, nt - c0)
                nc.sync.dma_start(kb_next[:, 0:nb * 512],
                                  k8_d[:, c0 * 512:(c0 + nb) * 512])
                nc.sync.dma_start(qrb_next[:, 0:nb * 512],
                                  q8_d[:, c0 * 512:(c0 + nb) * 512])
            if t % kch == 0 and t > 0:
                kb = kb_next
                qrb = qrb_next
            kt = kb[:, (t % kch) * 512:(t % kch) * 512 + 512]
            qt = qrb[:, (t % kch) * 512:(t % kch) * 512 + 512]

            if ti % 4 == 0:
                n4 = min(4, gt - ti)
                hp_t = pp_hp.tile([128, 512], f32, tag="hp")
                nc.tensor.matmul(hp_t[:, 0:n4 * 128], lT1,
                                 pb[:, ti * 128:(ti + n4) * 128],
                                 start=True, stop=True,
                                 tile_position=(0, 0), skip_group_check=True)
                hpos4 = spool.tile([128, 512], bf16, tag="hpos4")
                nc.scalar.activation(hpos4[:, 0:n4 * 128], hp_t[:, 0:n4 * 128],
                                     Relu)
            hpos = hpos4[:, (ti % 4) * 128:(ti % 4) * 128 + 128]

            def flush(tdone):
                pg = tdone // grp
                if tdone % grp == grp - 1 or tdone == nt - 1:
                    pgt = min(grp, nt - pg * grp)
                    p0 = pg * grp
                    nc.sync.dma_start(
                        outT_d[:, p0 * 64:(p0 + pgt) * 64],
                        obs.pop(pg)[:, 0:pgt * 64])

            cur = (t, (ti, obs[g], *front(t, kt, qt, hpos)))
            pq.append(cur)
            # tail1 at distance 2, tail2 at distance 4
            if len(pq) >= 4:
                td, args = pq[-4]
                if len(args) == 5:
                    pq[-4] = (td, tail1(*args))
            if len(pq) >= 7:
                td, args = pq.pop(0)
                tail2(*args)
                flush(td)
         while pq:
            td, args = pq.pop(0)
            if len(args) == 5:
                args = tail1(*args)
            tail2(*args)
            flush(td)


# ----------------------------------------------------------------------------
# entry point
# ----------------------------------------------------------------------------

def kernel(q, k, pos, strength, q_tbl, k_tbl, v_tbl,
           pos_w1, pos_b1, pos_w2, pos_b2,
           attn_w1, attn_b1, attn_w2, attn_b2,
           out_w, out_b, str_w, str_b, mask, embed_id1) -> np.ndarray:
    from concourse.bass_utils import run_bass_kernel_spmd

    inputs = dict(q=q, k=k, pos=pos, strength=strength, q_tbl=q_tbl,
                  k_tbl=k_tbl, v_tbl=v_tbl, pos_w1=pos_w1, pos_b1=pos_b1,
                  pos_w2=pos_w2, pos_b2=pos_b2, attn_w1=attn_w1,
                  attn_b1=attn_b1, attn_w2=attn_w2, attn_b2=attn_b2,
                  out_w=out_w, out_b=out_b, str_w=str_w, str_b=str_b,
                  mask=mask, embed_id1=embed_id1)
    nc = build_program(NT_FULL)
    consts, biasf = make_consts(inputs)
    mask_f = np.asarray(mask).astype(np.float32)
    in_maps = []
    for c in range(NCORES):
        m = prep_core(inputs["q"], inputs["k"], inputs["pos"], mask_f, c)
        m["consts"] = consts
        m["biasf"] = biasf
        in_maps.append(m)
    res = run_bass_kernel_spmd(nc, in_maps, core_ids=list(range(NCORES)))
    out = np.empty((B * N, DIM), np.float32)
    for c in range(NCORES):
        out[c * R_C:(c + 1) * R_C] = unprep_out(res.results[c]["outT"])
    return out.reshape(B, N, DIM)


# revision 5
# speedup vs baseline: 1.7809x; 1.0050x over previous
"""Trainium2 Bass kernel for nn_Attention2D (sparse_attention), v2.

Data-parallel over rays across 8 cores. Per core, 64 tiles of 1024
view-tokens. All activations feature-major. Within a tile, tokens are
grouped into 8 blocks of 128 tokens (16 rays x 8 views); blocks are paired
(pair i = blocks 2i, 2i+1) so K=64-deep matmuls can stack two blocks into
the 128-partition dim (block-diagonal lhsT), halving the charged output
free size versus a flat layout.

Math (validated in sim_check.py, rel err 6e-7 fp32):
  s cancels in kh - qh; with A_k = Wk.T@attn_w1, A_q = Wq.T@attn_w1,
  P_a = pos_w2@attn_w1, c_z = pos_b2@attn_w1 + attn_b1 the attn-MLP hidden is
  h1 = relu(k@A_k - q@A_q + hpos@P_a + c_z) with hpos = relu(pos@w1+b1).
  Mask rides as hpos channel 8 (ones as channel 9); masked tokens get h1
  clipped to 0 via +CLIP*(m-1), logits get +M_SHIFT*(m-1) so masked
  e = exp(b2-50) ~ 2e-22 (all-masked rays stay uniform). u = k@Wv.T +
  hpos@pos_w2 (its bias s+pos_b2 folds into the output bias).
  x = (sum_v u*e)/(sum_v e); out = x@out_w + b_out'.

Softmax tail: exp on Act; u*e split DVE/GpSimd; the two view-sums run as one
fused pairwise add-tree over the concatenated [e|u*e] fp16 tile (TT ops get
the 2x DVE mode); reciprocal_approx_fast + mult finish.
"""

import numpy as np
import ml_dtypes

BF16 = ml_dtypes.bfloat16
FP16 = np.float16
DIM, HID, B, N, V = 64, 8, 1024, 64, 8
NCORES = 8
B_C = B // NCORES
R_C = B_C * N              # 8192 rays per core
T_C = R_C * V              # 65536 view-tokens per core
TILE = 1024
NT_FULL = T_C // TILE      # 64 tiles
GRP = 16                   # tiles per grouped q/posm/out DMA
KCH = 8                    # tiles per k8 DMA chunk
M_SHIFT = 8.0
CLIP = 8.0
MULT_D = 224               # u*e columns done on DVE (rest on GpSimd)

# consts tensor column layout ([128, CW] bf16)
C_ST1, C_ZP, C_ZK, C_ZQ, C_W3, C_UP, C_WV, C_OW = (
    0, 128, 256, 288, 320, 832, 1344, 1472)
CW = 1600

_PROG_CACHE: dict = {}


def _f32(x):
    return np.ascontiguousarray(np.asarray(x), dtype=np.float32)


# ----------------------------------------------------------------------------
# host-side preparation
# ----------------------------------------------------------------------------

def make_consts(inputs):
    """[128, CW] bf16 consts + [128, 2] f32 biases (b_exp, b_out)."""
    eid = int(np.asarray(inputs["embed_id1"]))
    Wq = _f32(inputs["q_tbl"])[eid].reshape(DIM, DIM)
    Wk = _f32(inputs["k_tbl"])[eid].reshape(DIM, DIM)
    Wv = _f32(inputs["v_tbl"])[eid].reshape(DIM, DIM)
    pos_w1, pos_b1 = _f32(inputs["pos_w1"]), _f32(inputs["pos_b1"])
    pos_w2, pos_b2 = _f32(inputs["pos_w2"]), _f32(inputs["pos_b2"])
    attn_w1, attn_b1 = _f32(inputs["attn_w1"]), _f32(inputs["attn_b1"])
    attn_w2, attn_b2 = _f32(inputs["attn_w2"]), _f32(inputs["attn_b2"])
    out_w, out_b = _f32(inputs["out_w"]), _f32(inputs["out_b"])

    s = _f32(inputs["strength"]) @ _f32(inputs["str_w"]) + _f32(inputs["str_b"])
    A_k = Wk.T @ attn_w1                          # [64, 8]
    A_q = Wq.T @ attn_w1
    P_a = pos_w2 @ attn_w1                        # [8, 8]
    c_z = pos_b2 @ attn_w1 + attn_b1              # [8]
    b_out = (s + pos_b2) @ out_w + out_b          # [64]

    C = np.zeros((128, CW), np.float32)
    # stage-1 lhsT [48, 128]: 8 diag blocks of [6, 16]
    for b in range(8):
        r, c = 6 * b, 16 * b
        C[r:r + 4, C_ST1 + c:C_ST1 + c + 8] = pos_w1
        C[r + 5, C_ST1 + c:C_ST1 + c + 8] = pos_b1
        C[r + 4, C_ST1 + c + 8] = 1.0             # mask carry
        C[r + 5, C_ST1 + c + 9] = 1.0             # ones carry
    # zp lhsT [128, 128]: 8 diag blocks of [16, 16]
    for b in range(8):
        r = 16 * b
        C[r:r + 8, C_ZP + r:C_ZP + r + 8] = P_a
        C[r + 8, C_ZP + r:C_ZP + r + 8] = CLIP
        C[r + 8, C_ZP + r + 8] = 1.0
        C[r + 9, C_ZP + r:C_ZP + r + 8] = c_z - CLIP
        C[r + 9, C_ZP + r + 9] = 1.0
    # z1k / zq lhsT [128, 32]: 2 diag blocks of [64, 16]
    for sub in range(2):
        r, c = 64 * sub, 16 * sub
        C[r:r + 64, C_ZK + c:C_ZK + c + 8] = A_k
        C[r:r + 64, C_ZQ + c:C_ZQ + c + 8] = -A_q
    # W3 / UP lhsT: 4 K-padded [128, 128] blocks (pair i rows at 32i)
    for i in range(4):
        for sub in range(2):
            r, c = 32 * i + 16 * sub, 128 * i + 64 * sub
            C[r:r + 8, C_W3 + c:C_W3 + c + 64] = attn_w2
            C[r + 8, C_W3 + c:C_W3 + c + 64] = M_SHIFT
            C[r + 9, C_W3 + c:C_W3 + c + 64] = attn_b2
            C[r:r + 8, C_UP + c:C_UP + c + 64] = pos_w2
    # Wv.T / out_w [128, 128]: 2 diag blocks of [64, 64]
    for sub in range(2):
        r = 64 * sub
        C[r:r + 64, C_WV + r:C_WV + r + 64] = Wv.T
        C[r:r + 64, C_OW + r:C_OW + r + 64] = out_w

    biasf = np.zeros((128, 2), np.float32)
    biasf[:, 0] = -M_SHIFT
    for sub in range(2):
        biasf[64 * sub:64 * sub + 64, 1] = b_out
    return C.astype(BF16), biasf


def prep_core(q, k, pos, mask_f, core):
    """Per-core transposed contiguous arrays (bf16)."""
    b0 = core * B_C
    kc = _f32(k[b0:b0 + B_C]).reshape(T_C, DIM).astype(BF16)
    qc = _f32(q[b0:b0 + B_C]).reshape(R_C, DIM).astype(BF16)
    pc = _f32(pos[b0:b0 + B_C]).reshape(T_C, 4)
    mc = mask_f[b0:b0 + B_C].reshape(T_C)

    # k8[sub*64+ch, t*512 + i*128 + j] = k[t*1024 + (2i+sub)*128 + j, ch]
    k8 = np.ascontiguousarray(
        kc.reshape(NT_FULL, 4, 2, 128, DIM).transpose(2, 4, 0, 1, 3)
        .reshape(128, NT_FULL * 512))
    # q8r[sub*64+ch, t*512 + i*128 + jr*8 + v] = q[t*128 + (2i+sub)*16 + jr, ch]
    q8 = qc.reshape(NT_FULL, 4, 2, 16, DIM).transpose(2, 4, 0, 1, 3)
    q8 = np.ascontiguousarray(np.broadcast_to(
        q8.reshape(2, DIM, NT_FULL, 4, 16, 1),
        (2, DIM, NT_FULL, 4, 16, V)).reshape(128, NT_FULL * 512))
    # posm[b*6+e, t*128 + j]: e 0-3 pos, 4 mask, 5 ones
    pm = np.empty((8, 6, NT_FULL, 128), np.float32)
    pm[:, 0:4] = pc.reshape(NT_FULL, 8, 128, 4).transpose(1, 3, 0, 2)
    pm[:, 4] = mc.reshape(NT_FULL, 8, 128).transpose(1, 0, 2)
    pm[:, 5] = 1.0
    posm = np.ascontiguousarray(pm.reshape(48, NT_FULL * 128).astype(BF16))
    return {"k8": k8, "q8": q8, "posm": posm}


def unprep_out(outT):
    """outT [128, NT*64] -> [R_C, 64] ray-major."""
    v = outT.reshape(2, DIM, NT_FULL, 4, 16).transpose(2, 3, 0, 4, 1)
    return np.ascontiguousarray(v.reshape(R_C, DIM))


# ----------------------------------------------------------------------------
# device program
# ----------------------------------------------------------------------------

def build_program(nt=NT_FULL, nrep=1):
    if (nt, nrep) in _PROG_CACHE:
        return _PROG_CACHE[(nt, nrep)]

    import concourse.bacc as bacc
    import concourse.tile as tile
    import concourse.mybir as mybir

    f32 = mybir.dt.float32
    bf16 = mybir.dt.bfloat16
    nc = bacc.Bacc("TRN2", target_bir_lowering=False, debug=False,
                   enable_asserts=False, num_devices=NCORES)
    k8_d = nc.dram_tensor("k8", [128, nt * 512], bf16, kind="ExternalInput").ap()
    q8_d = nc.dram_tensor("q8", [128, nt * 512], bf16, kind="ExternalInput").ap()
    posm_d = nc.dram_tensor("posm", [48, nt * 128], bf16, kind="ExternalInput").ap()
    cons_d = nc.dram_tensor("consts", [128, CW], bf16, kind="ExternalInput").ap()
    bias_d = nc.dram_tensor("biasf", [128, 2], f32, kind="ExternalInput").ap()
    outT_d = nc.dram_tensor("outT", [128, nt * 64], f32, kind="ExternalOutput").ap()

    with tile.TileContext(nc) as tc:
        _emit(tc, nc, mybir, k8_d, q8_d, posm_d, cons_d, bias_d, outT_d, nt, nrep)
    nc.compile()
    _PROG_CACHE[(nt, nrep)] = nc
    return nc


def _emit(tc, nc, mybir, k8_d, q8_d, posm_d, cons_d, bias_d, outT_d, nt, nrep=1):
    from contextlib import ExitStack

    f32 = mybir.dt.float32
    bf16 = mybir.dt.bfloat16
    fp16 = mybir.dt.float16
    Exp = mybir.ActivationFunctionType.Exp
    Copy = mybir.ActivationFunctionType.Copy
    Relu = mybir.ActivationFunctionType.Relu
    Ident = mybir.ActivationFunctionType.Identity
    add = mybir.AluOpType.add
    mult = mybir.AluOpType.mult
    grp = min(GRP, nt)
    kch = min(KCH, nt)

    with ExitStack() as ctx:
        ep = ctx.enter_context
        cpool = ep(tc.tile_pool(name="consts", bufs=1))
        kpool = ep(tc.tile_pool(name="kt", bufs=3))
        qpool = ep(tc.tile_pool(name="qt", bufs=3))
        pmpool = ep(tc.tile_pool(name="pm", bufs=2))
        spool = ep(tc.tile_pool(name="sb", bufs=8))
        tpool = ep(tc.tile_pool(name="tree", bufs=8))
        xpool = ep(tc.tile_pool(name="xs", bufs=8))
        opool = ep(tc.tile_pool(name="ob", bufs=2))
        pp_hp = ep(tc.tile_pool(name="ps_hp", bufs=1, space="PSUM"))
        pp_z = ep(tc.tile_pool(name="ps_z", bufs=1, space="PSUM"))
        pp_lg = ep(tc.tile_pool(name="ps_lg", bufs=1, space="PSUM"))
        pp_o = ep(tc.tile_pool(name="ps_o", bufs=1, space="PSUM"))
        pp_u = ep(tc.tile_pool(name="ps_u", bufs=4, space="PSUM"))

        cons = cpool.tile([128, CW], bf16, tag="consts")
        nc.sync.dma_start(cons[:], cons_d[:, :])
        biasf = cpool.tile([128, 2], f32, tag="biasf")
        nc.sync.dma_start(biasf[:], bias_d[:, :])
        b_exp = biasf[:, 0:1]
        b_out = biasf[:, 1:2]

        lT1 = cons[0:48, C_ST1:C_ST1 + 128]
        lZP = cons[:, C_ZP:C_ZP + 128]
        lZK = cons[:, C_ZK:C_ZK + 32]
        lZQ = cons[:, C_ZQ:C_ZQ + 32]
        lWV = cons[:, C_WV:C_WV + 128]
        lOW = cons[:, C_OW:C_OW + 128]

        for rep in range(nrep):
         def front(t, kt, qt, hpos):
            """Stages up to exp: returns (lg, u, et) tiles for the tail."""
            z8_t = pp_z.tile([128, 128], f32, tag="z8")
            z8 = z8_t[:]
            for i in range(4):
                nc.tensor.matmul(
                    z8[32 * i:32 * i + 32, :], lZK,
                    kt[:, 128 * i:128 * i + 128], start=True, stop=False,
                    tile_position=(0, 32 * i), skip_group_check=True)
            for i in range(4):
                nc.tensor.matmul(
                    z8[32 * i:32 * i + 32, :], lZQ,
                    qt[:, 128 * i:128 * i + 128], start=False, stop=False,
                    tile_position=(0, 32 * i), skip_group_check=True)
            nc.tensor.matmul(z8, lZP, hpos, start=False, stop=True,
                             tile_position=(0, 0), skip_group_check=True)
            h1 = spool.tile([128, 128], bf16, tag="h1")
            nc.vector.tensor_scalar_max(h1[:], z8, 0.0)

            lg = pp_lg.tile([128, 512], f32, tag="lg")
            for i in range(4):
                nc.tensor.matmul(
                    lg[:, 128 * i:128 * i + 128],
                    cons[:, C_W3 + 128 * i:C_W3 + 128 * i + 128],
                    h1[:], start=True, stop=True,
                    tile_position=(0, 0), skip_group_check=True)

            u = pp_u.tile([128, 512], f32, tag="u")
            nc.tensor.matmul(u[:], lWV, kt, start=True, stop=False,
                             tile_position=(0, 0), skip_group_check=True)
            for i in range(4):
                nc.tensor.matmul(
                    u[:, 128 * i:128 * i + 128],
                    cons[:, C_UP + 128 * i:C_UP + 128 * i + 128],
                    hpos, start=False, stop=True,
                    tile_position=(0, 0), skip_group_check=True)

            et = spool.tile([128, 1024], fp16, tag="et")
            nc.scalar.activation(et[:, 0:512], lg[:], Exp, bias=b_exp)
            return lg, u, et

         def tail1(ti_ob, ob, lg, u, et):
            with nc.allow_low_precision(reason="fp16 softmax tail"):
                nc.vector.tensor_tensor(
                    et[:, 512:1024], u[:, 0:512], et[:, 0:512], mult)
                etv = et[:].rearrange("p (gp v) -> p gp v", v=8)
                t1 = tpool.tile([128, 512], fp16, tag="t1")
                t1v = t1[:].rearrange("p (gp v) -> p gp v", v=4)
                nc.vector.tensor_tensor(t1v, etv[:, :, 0:4], etv[:, :, 4:8], add)
                t2 = tpool.tile([128, 256], fp16, tag="t2")
                t2v = t2[:].rearrange("p (gp v) -> p gp v", v=2)
                nc.gpsimd.tensor_tensor(t2v, t1v[:, :, 0:2], t1v[:, :, 2:4], add)
                dn = tpool.tile([128, 128], f32, tag="dn")
                dnv = dn[:].rearrange("p (gp v) -> p gp v", v=1)
                nc.gpsimd.tensor_tensor(dnv, t2v[:, :, 0:1], t2v[:, :, 1:2], add)
            return (ti_ob, ob, dn)

         def tail2(ti_ob, ob, dn):
            rden = xpool.tile([128, 64], f32, tag="rden")
            nc.vector.reciprocal_approx_fast(rden[:], dn[:, 0:64])
            x = xpool.tile([128, 64], bf16, tag="x")
            nc.gpsimd.tensor_tensor(x[:], dn[:, 64:128], rden[:], mult)

            o_t = pp_o.tile([128, 64], f32, tag="o")
            o_ps = o_t[:]
            nc.tensor.matmul(o_ps, lOW, x[:], start=True, stop=True,
                             tile_position=(0, 0), skip_group_check=True)
            nc.scalar.activation(ob[:, ti_ob * 64:ti_ob * 64 + 64], o_ps,
                                 Ident, bias=b_out)

         # software-pipelined main loop: front(t+1) is emitted before tail(t)
         pq = []                 # [(t, front-result or tail1-result)]
         kb = None
         obs = {}
         for t in range(nt):
            g, ti = t // grp, t % grp
            if ti == 0:
                gt = min(grp, nt - g * grp)
                t0 = g * grp
                pass
                pb = pmpool.tile([48, grp * 128], bf16, tag="pb")
                nc.sync.dma_start(pb[:, 0:gt * 128],
                                  posm_d[:, t0 * 128:(t0 + gt) * 128])
                obs[g] = opool.tile([128, grp * 64], f32, tag="ob", name=f"ob{g}")
            if t == 0:
                kb = kpool.tile([128, kch * 512], bf16, tag="kb")
                nc.sync.dma_start(kb[:, 0:min(kch, nt) * 512],
                                  k8_d[:, 0:min(kch, nt) * 512])
                qrb = qpool.tile([128, kch * 512], bf16, tag="qrb")
                nc.sync.dma_start(qrb[:, 0:min(kch, nt) * 512],
                                  q8_d[:, 0:min(kch, nt) * 512])
                kb_next = None
                qrb_next = None
            if t % kch == 1 and t + kch - 1 < nt:
                kb_next = kpool.tile([128, kch * 512], bf16, tag="kb")
                qrb_next = qpool.tile([128, kch * 512], bf16, tag="qrb")
                c0 = (t // kch + 1) * kch
                nb = min(kch, nt - c0)
                nc.sync.dma_start(kb_next[:, 0:nb * 512],
                                  k8_d[:, c0 * 512:(c0 + nb) * 512])
                nc.sync.dma_start(qrb_next[:, 0:nb * 512],
                                  q8_d[:, c0 * 512:(c0 + nb) * 512])
            if t % kch == 0 and t > 0:
                kb = kb_next
                qrb = qrb_next
            kt = kb[:, (t % kch) * 512:(t % kch) * 512 + 512]
            qt = qrb[:, (t % kch) * 512:(t % kch) * 512 + 512]

            if ti % 4 == 0:
                n4 = min(4, gt - ti)
                hp_t = pp_hp.tile([128, 512], f32, tag="hp")
                nc.tensor.matmul(hp_t[:, 0:n4 * 128], lT1,
                                 pb[:, ti * 128:(ti + n4) * 128],
                                 start=True, stop=True,
                                 tile_position=(0, 0), skip_group_check=True)
                hpos4 = spool.tile([128, 512], bf16, tag="hpos4")
                nc.scalar.activation(hpos4[:, 0:n4 * 128], hp_t[:, 0:n4 * 128],
                                     Relu)
            hpos = hpos4[:, (ti % 4) * 128:(ti % 4) * 128 + 128]

            def flush(tdone):
                pg = tdone // grp
                if tdone % grp == grp - 1 or tdone == nt - 1:
                    pgt = min(grp, nt - pg * grp)
                    p0 = pg * grp
                    nc.sync.dma_start(
                        outT_d[:, p0 * 64:(p0 + pgt) * 64],
                        obs.pop(pg)[:, 0:pgt * 64])

            cur = (t, (ti, obs[g], *front(t, kt, qt, hpos)))
            pq.append(cur)
            # tail1 at distance 2, tail2 at distance 4
            if len(pq) >= 4:
                td, args = pq[-4]
                if len(args) == 5:
                    pq[-4] = (td, tail1(*args))
            if len(pq) >= 7:
                td, args = pq.pop(0)
                tail2(*args)
                flush(td)
         while pq:
            td, args = pq.pop(0)
            if len(args) == 5:
                args = tail1(*args)
            tail2(*args)
            flush(td)


# ----------------------------------------------------------------------------
# entry point
# ----------------------------------------------------------------------------

def kernel(q, k, pos, strength, q_tbl, k_tbl, v_tbl,
           pos_w1, pos_b1, pos_w2, pos_b2,
           attn_w1, attn_b1, attn_w2, attn_b2,
           out_w, out_b, str_w, str_b, mask, embed_id1) -> np.ndarray:
    from concourse.bass_utils import run_bass_kernel_spmd

    inputs = dict(q=q, k=k, pos=pos, strength=strength, q_tbl=q_tbl,
                  k_tbl=k_tbl, v_tbl=v_tbl, pos_w1=pos_w1, pos_b1=pos_b1,
                  pos_w2=pos_w2, pos_b2=pos_b2, attn_w1=attn_w1,
                  attn_b1=attn_b1, attn_w2=attn_w2, attn_b2=attn_b2,
                  out_w=out_w, out_b=out_b, str_w=str_w, str_b=str_b,
                  mask=mask, embed_id1=embed_id1)
    nc = build_program(NT_FULL)
    consts, biasf = make_consts(inputs)
    mask_f = np.asarray(mask).astype(np.float32)
    in_maps = []
    for c in range(NCORES):
        m = prep_core(inputs["q"], inputs["k"], inputs["pos"], mask_f, c)
        m["consts"] = consts
        m["biasf"] = biasf
        in_maps.append(m)
    res = run_bass_kernel_spmd(nc, in_maps, core_ids=list(range(NCORES)))
    out = np.empty((B * N, DIM), np.float32)
    for c in range(NCORES):
        out[c * R_C:(c + 1) * R_C] = unprep_out(res.results[c]["outT"])
    return out.reshape(B, N, DIM)
